# revision 30
# baseline (speedup 1.0000x reference)
"""Sparse-attention kernel for Trainium2 (8 NeuronCores, data-parallel over batch).

Reference computation (L=2048, B=128, H=300):
    proj[l,b,k]   = sum_h qv[l,b,h] * W[k,h] + bias[k]
    energies[b,l] = sum_k proj[l,b,k] * hidden[k,b]
    attn          = softmax(energies, axis=-1)[None]

Algebraic reduction:
    energies[b,l] = sum_h qv[l,b,h] * Wh[h,b],  Wh = W^T @ hidden
(the bias term is constant over l and cancels in the softmax).

This version is PE-centric.  The host pre-transposes each core's qv slice
to a [4800 (b,h) rows, L cols] fp16 matrix (37 full 128-row strips plus a
64-row strip) and builds block-sparse [<=128, 16] fp16 stationaries Wst
with Wst[s][q, b] = Wh[h, b] iff row 128*s+q == b*300+h.  For each
256-column tile of L, the energy block E[b, lt] = sum_s Wst[s]^T @
qvT[strip s, lt] is a 38-matmul PSUM accumulation group on the tensor
engine: the multiply and the h-reduction both happen inside the PE, and
the result lands already transposed ([16 batches, L]) for the softmax.
Each tile's DMA is issued as four strip-quarters (+ the 64-row piece) on
separate semaphores so the PE trails the transfer by only ~10 strips.
ACT exponentiates each tile out of PSUM into bf16 (a per-batch shift
estimated from ||Wh[:, b]|| centres the row max near 1; bf16's f32-like
exponent range absorbs the +-10 estimate error, and the shift cancels
exactly in the normalisation).  Row sums accumulate per tile via
accum_out; the tail is one tiny reduce + reciprocal + a DVE/ACT split
rescale with the two output halves DMA'd out as they finish.  fp16 data
halves the HBM traffic, which is the roofline for this memory-bound
problem (DMA busy ~55.4us of the ~64.5us total); PE fp16 matmuls
contract with fp32 PSUM accumulation so energies stay accurate to ~4e-3.

Raw Bass (manual semaphores): the walrus codegen used by the axon path
rejects Tile's multi-wait instructions, custom ISA ops (tensor scans,
tensor_tensor_reduce) and all Pool-engine compute, so everything is
standard DMA/PE/ACT/DVE instructions with standalone wait_ge.
"""

import sys

if "/opt/trn_rl_repo" not in sys.path:
    sys.path.insert(0, "/opt/trn_rl_repo")

import numpy as np

L, B, H = 2048, 128, 300
NCORES = 8
BL = B // NCORES          # 16 batches per core
RROWS = BL * H            # 4800 (b,h) rows
NSTRIP = (RROWS + 127) // 128  # 38 strips (last strip has 64 rows)
LAST_ROWS = RROWS - (NSTRIP - 1) * 128  # 64
# Per-batch softmax shift: energies e[b, :] have std sigma_b = ||Wh[:, b]||
# (qv is unit-variance), so the row max over 2048 samples is ~3.9*sigma_b.
# exp(e - m_b) with m_b = 3.9*sigma_b + 4 keeps the hot entries inside
# fp16 range (overflow needs e > m_b + 11.1, underflow flushes only
# entries >16.6 below m_b, whose softmax weight is < 6e-8).  The shift is
# per-row constant so it cancels exactly in the normalization.
MSCALE, MOFF = 3.9, 4.0

# L tiling: 8 tiles of 256 (descriptors stay >= 512B for full DMA rate).
# Each tile's DMA is issued as two strip-halves so the PE can start on the
# first 19 strips while the rest are still in flight.
TILES = [(d * 256, 256) for d in range(8)]
NT = len(TILES)
NSLOT = 4                 # qt tile buffers
TW = 256                  # slot width
# strip ranges per DMA quarter (last quarter also covers the 64-row strip)
QUARTERS = [(0, 11), (11, 21), (21, 31), (31, NSTRIP - 1)]

_cache = {}


def _build_nc():
    import concourse.bass as bass
    from concourse import mybir

    f16 = mybir.dt.float16
    f32 = mybir.dt.float32
    Alu = mybir.AluOpType
    Act = mybir.ActivationFunctionType

    nc = bass.Bass("TRN2", target_bir_lowering=False, debug=False)

    qvt_h = nc.dram_tensor("qvt", [RROWS, L], f16, kind="ExternalInput")
    wst_d = nc.dram_tensor("wst", [128, NSTRIP * BL], f16, kind="ExternalInput").ap()
    nmx_d = nc.dram_tensor("nmx", [BL, 1], f32, kind="ExternalInput").ap()
    out = nc.dram_tensor("out", [BL, L], f32, kind="ExternalOutput").ap()

    # --- SBUF
    wst = nc.alloc_sbuf_tensor("wst_t", [128, NSTRIP * BL], f16).ap()
    qth = [nc.alloc_sbuf_tensor(f"qt{s}", [128, NSTRIP * TW], f16) for s in range(NSLOT)]
    qt = [h.ap() for h in qth]
    bf16 = mybir.dt.bfloat16
    xT = nc.alloc_sbuf_tensor("xT", [BL, L], bf16).ap()
    ssp = nc.alloc_sbuf_tensor("ssp", [BL, NT], f32).ap()
    ssum = nc.alloc_sbuf_tensor("ssum", [BL, 1], f32).ap()
    rs = nc.alloc_sbuf_tensor("rs", [BL, 1], f32).ap()
    aT = nc.alloc_sbuf_tensor("aT", [BL, L], f32).ap()
    nmx = nc.alloc_sbuf_tensor("nmx_t", [BL, 1], f32).ap()

    # --- PSUM: two rotating energy banks
    ep = [nc.psum_tensor(f"ep{i}", [BL, TW], f32).__enter__().ap() for i in range(2)]

    # --- semaphores
    SW = nc.alloc_semaphore("SW")      # wst load
    SQQ = [[nc.alloc_semaphore(f"SQ{q}_{s}") for s in range(NSLOT)]
           for q in range(4)]
    SQE = [nc.alloc_semaphore(f"SQe{s}") for s in range(NSLOT)]
    SQF = nc.alloc_semaphore("SQf")  # last tile: strips 34..37
    SMM = nc.alloc_semaphore("SMM")    # PE tile done (1 per tile)
    SX = nc.alloc_semaphore("SX")      # ACT exp tile done
    SC = nc.alloc_semaphore("SC")      # nmx ready
    SRS = nc.alloc_semaphore("SRS")    # reciprocal ready
    SFIN = nc.alloc_semaphore("SFIN")   # DVE rescale half done
    SFIN2 = nc.alloc_semaphore("SFIN2")  # ACT rescale half done
    SNG = nc.alloc_semaphore("SNG")    # DVE same-engine ordering
    SOUT = nc.alloc_semaphore("SOUT")  # output DMA landed
    all_sems = [SW, *[s for qq in SQQ for s in qq], *SQE, SQF, SMM, SX, SC, SRS,
                SFIN, SFIN2, SNG, SOUT]
    sem_final = {s.name: 0 for s in all_sems}

    def inc(inst, sem, n=1):
        sem_final[sem.name] += n
        return inst.then_inc(sem, n)

    # DMA pattern for strips [s0, s1) of a tile: SBUF (partition q, strip s)
    # holds qvt row 128*s+q, cols l0..l0+w.
    def qv_tile_in(l0, w, s0, s1):
        return bass.AP(qvt_h, s0 * 128 * L + l0,
                       [[L, 128], [128 * L, s1 - s0], [1, w]])

    def emit_tile_dmas(sync, t, first=False):
        l0, w = TILES[t]
        s = t % NSLOT
        # the tiny 64-row strip goes FIRST so the tile's last transfer is a
        # regular quarter (shifts the stream end earlier for the final tile)
        inc(sync.dma_start(
            out=bass.AP(qth[s], (NSTRIP - 1) * w, [[NSTRIP * w, LAST_ROWS], [1, w]]),
            in_=bass.AP(qvt_h, (NSTRIP - 1) * 128 * L + l0, [[L, LAST_ROWS], [1, w]]),
        ), SQE[s], 16)
        quarters = QUARTERS if t < NT - 1 else QUARTERS[:3] + [(31, 35)]
        for q, (s0, s1) in enumerate(quarters):
            inc(sync.dma_start(
                out=qt[s][:, s0 * w : s1 * w],
                in_=qv_tile_in(l0, w, s0, s1),
            ), SQQ[q][s], 16)
            if first and q == 0:
                # stationaries + shifts ride behind the first quarter so the
                # first tile's data leads the queue
                inc(sync.dma_start(out=wst, in_=wst_d), SW, 16)
                inc(sync.dma_start(out=nmx, in_=nmx_d), SC, 16)
        if t == NT - 1:
            inc(sync.dma_start(
                out=qt[s][:, 35 * w : 37 * w],
                in_=qv_tile_in(l0, w, 35, 37),
            ), SQF, 16)

    with nc.Block() as block:

        @block.sync
        def _(sync):
            # first tiles up front, then stream with slot reuse
            for t in range(NSLOT):
                emit_tile_dmas(sync, t, first=(t == 0))
            for t in range(NSLOT, NT):
                sync.wait_ge(SMM, t - NSLOT + 1)  # PE done with slot t%NSLOT
                emit_tile_dmas(sync, t)
            # stream the two rescaled output halves out as they finish
            sync.wait_ge(SFIN, 1)
            inc(sync.dma_start(out=out[:, :960], in_=aT[:, :960]), SOUT, 16)
            sync.wait_ge(SFIN2, 1)
            inc(sync.dma_start(out=out[:, 960:], in_=aT[:, 960:]), SOUT, 16)

        @block.tensor
        def _(pe):
            pe.wait_ge(SW, 16)
            for t in range(NT):
                l0, w = TILES[t]
                p = t // NSLOT
                if t >= 2:
                    pe.wait_ge(SX, t - 1)  # exp done with this ep bank
                for s in range(NSTRIP):
                    for q, (s0, s1) in enumerate(QUARTERS):
                        if s == s0:
                            pe.wait_ge(SQQ[q][t % NSLOT], 16 * p + 16)
                    if t == NT - 1 and s == 35:
                        pe.wait_ge(SQF, 16)
                    if s == NSTRIP - 1:
                        pe.wait_ge(SQE[t % NSLOT], 16 * p + 16)
                    rows = 128 if s < NSTRIP - 1 else LAST_ROWS
                    mm = pe.matmul(
                        ep[t % 2][:, :w],
                        wst[0:rows, s * BL : (s + 1) * BL],
                        bass.AP(qth[t % NSLOT], s * w, [[NSTRIP * w, rows], [1, w]]),
                        start=(s == 0),
                        stop=(s == NSTRIP - 1),
                    )
                inc(mm, SMM)

        @block.scalar
        def _(act):
            act.wait_ge(SC, 16)  # nmx loaded
            for t in range(NT):
                l0, w = TILES[t]
                act.wait_ge(SMM, t + 1)
                inc(act.activation(
                    xT[:, l0 : l0 + w], ep[t % 2][:, :w], Act.Exp,
                    bias=nmx, scale=1.0, accum_out=ssp[:, t : t + 1],
                ), SX)
            # tail: rescale second half once rs is ready
            act.wait_ge(SRS, 1)
            inc(act.mul(aT[:, 960:], xT[:, 960:], rs), SFIN2)

        @block.vector
        def _(dve):
            dve.wait_ge(SX, NT)  # all tiles exponentiated
            inc(dve.tensor_reduce(out=ssum, in_=ssp, axis=mybir.AxisListType.X,
                                  op=Alu.add), SNG)
            dve.wait_ge(SNG, 1)  # DVE deep pipeline: order ssum -> reciprocal
            inc(dve.reciprocal(rs, ssum), SRS)
            dve.wait_ge(SRS, 1)  # order rs -> rescale read
            inc(dve.tensor_scalar(out=aT[:, :960], in0=xT[:, :960],
                                  scalar1=rs, scalar2=None, op0=Alu.mult), SFIN)


        @block.gpsimd
        def _(gp):
            gp.wait_ge(SOUT, 32)

        nc.all_engine_barrier()
        for s in all_sems:
            if sem_final[s.name]:
                nc.gpsimd.sem_inc(s, -sem_final[s.name])

    return nc


def _get_nc():
    if "nc" not in _cache:
        _cache["nc"] = _build_nc()
    return _cache["nc"]


def make_in_maps(hidden, question_vector, W):
    hidden = np.asarray(hidden, dtype=np.float64)
    W = np.asarray(W, dtype=np.float64)
    qv = np.asarray(question_vector, dtype=np.float32)
    in_maps = []
    for i in range(NCORES):
        sl = slice(i * BL, (i + 1) * BL)
        wh = (W.T @ hidden[:, sl]).astype(np.float32)  # [H, BL]
        # block-sparse stationaries: wst[q, s*BL+b] = Wh[h, b] iff 128s+q = b*300+h
        wst = np.zeros((128, NSTRIP * BL), dtype=np.float16)
        r = np.arange(RROWS)
        bb, hh = r // H, r % H
        wst[r % 128, (r // 128) * BL + bb] = wh[hh, bb].astype(np.float16)
        sig = np.sqrt((wh.astype(np.float64) ** 2).sum(0))          # [BL]
        nmxv = -(MSCALE * sig + MOFF).astype(np.float32)[:, None]   # [BL, 1]
        # transposed qv: row (b,h), col l; padded to PROWS rows
        qs = qv[:, sl, :].astype(np.float16)           # [L, BL, H]
        qvt = qs.transpose(1, 2, 0).reshape(RROWS, L)
        in_maps.append({"qvt": np.ascontiguousarray(qvt), "wst": wst,
                        "nmx": np.ascontiguousarray(nmxv)})
    return in_maps


def kernel(hidden, question_vector, W, b=None, **kwargs):
    from concourse.bass_utils import run_bass_kernel_spmd

    nc = _get_nc()
    in_maps = make_in_maps(hidden, question_vector, W)
    res = run_bass_kernel_spmd(nc, in_maps, list(range(NCORES)))
    _cache["last_results"] = res
    outs = [np.asarray(res.results[i]["out"]) for i in range(NCORES)]
    attn = np.concatenate(outs, axis=0)[None]
    return np.ascontiguousarray(attn.astype(np.float32))


# revision 31
# speedup vs baseline: 1.0101x; 1.0101x over previous
"""Sparse-attention kernel for Trainium2 (8 NeuronCores, data-parallel over batch).

Reference computation (L=2048, B=128, H=300):
    proj[l,b,k]   = sum_h qv[l,b,h] * W[k,h] + bias[k]
    energies[b,l] = sum_k proj[l,b,k] * hidden[k,b]
    attn          = softmax(energies, axis=-1)[None]

Algebraic reduction:
    energies[b,l] = sum_h qv[l,b,h] * Wh[h,b],  Wh = W^T @ hidden
(the bias term is constant over l and cancels in the softmax).

This version is PE-centric.  The host pre-transposes each core's qv slice
to a [4800 (b,h) rows, L cols] fp16 matrix (37 full 128-row strips plus a
64-row strip) and builds block-sparse [<=128, 16] fp16 stationaries Wst
with Wst[s][q, b] = Wh[h, b] iff row 128*s+q == b*300+h.  For each
256-column tile of L, the energy block E[b, lt] = sum_s Wst[s]^T @
qvT[strip s, lt] is a 38-matmul PSUM accumulation group on the tensor
engine: the multiply and the h-reduction both happen inside the PE, and
the result lands already transposed ([16 batches, L]) for the softmax.
Each tile's DMA is issued as four strip-quarters (+ the 64-row piece) on
separate semaphores so the PE trails the transfer by only ~10 strips.
ACT exponentiates each tile out of PSUM into bf16 (a per-batch shift
estimated from ||Wh[:, b]|| centres the row max near 1; bf16's f32-like
exponent range absorbs the +-10 estimate error, and the shift cancels
exactly in the normalisation).  Row sums accumulate per tile via
accum_out; the tail is one tiny reduce + reciprocal + a DVE/ACT split
rescale with the two output halves DMA'd out as they finish.  fp16 data
halves the HBM traffic, which is the roofline for this memory-bound
problem (DMA busy ~55.4us of the ~64.5us total); PE fp16 matmuls
contract with fp32 PSUM accumulation so energies stay accurate to ~4e-3.

Raw Bass (manual semaphores): the walrus codegen used by the axon path
rejects Tile's multi-wait instructions, custom ISA ops (tensor scans,
tensor_tensor_reduce) and all Pool-engine compute, so everything is
standard DMA/PE/ACT/DVE instructions with standalone wait_ge.
"""

import sys

if "/opt/trn_rl_repo" not in sys.path:
    sys.path.insert(0, "/opt/trn_rl_repo")

import numpy as np

L, B, H = 2048, 128, 300
NCORES = 8
BL = B // NCORES          # 16 batches per core
RROWS = BL * H            # 4800 (b,h) rows
NSTRIP = (RROWS + 127) // 128  # 38 strips (last strip has 64 rows)
LAST_ROWS = RROWS - (NSTRIP - 1) * 128  # 64
# Per-batch softmax shift: energies e[b, :] have std sigma_b = ||Wh[:, b]||
# (qv is unit-variance), so the row max over 2048 samples is ~3.9*sigma_b.
# exp(e - m_b) with m_b = 3.9*sigma_b + 4 keeps the hot entries inside
# fp16 range (overflow needs e > m_b + 11.1, underflow flushes only
# entries >16.6 below m_b, whose softmax weight is < 6e-8).  The shift is
# per-row constant so it cancels exactly in the normalization.
MSCALE, MOFF = 3.9, 4.0

# L tiling: 8 tiles of 256 (descriptors stay >= 512B for full DMA rate).
# Each tile's DMA is issued as two strip-halves so the PE can start on the
# first 19 strips while the rest are still in flight.
TILES = [(d * 256, 256) for d in range(8)]
NT = len(TILES)
NSLOT = 4                 # qt tile buffers
TW = 256                  # slot width
# strip ranges per DMA quarter (last quarter also covers the 64-row strip)
QUARTERS = [(0, 11), (11, 21), (21, 31), (31, NSTRIP - 1)]

_cache = {}


def _build_nc():
    import concourse.bass as bass
    from concourse import mybir

    f16 = mybir.dt.float16
    f32 = mybir.dt.float32
    Alu = mybir.AluOpType
    Act = mybir.ActivationFunctionType

    nc = bass.Bass("TRN2", target_bir_lowering=False, debug=False)

    qvt_h = nc.dram_tensor("qvt", [RROWS, L], f16, kind="ExternalInput")
    wst_d = nc.dram_tensor("wst", [128, NSTRIP * BL], f16, kind="ExternalInput").ap()
    nmx_d = nc.dram_tensor("nmx", [BL, 1], f32, kind="ExternalInput").ap()
    out = nc.dram_tensor("out", [BL, L], f32, kind="ExternalOutput").ap()

    # --- SBUF
    wst = nc.alloc_sbuf_tensor("wst_t", [128, NSTRIP * BL], f16).ap()
    qth = [nc.alloc_sbuf_tensor(f"qt{s}", [128, NSTRIP * TW], f16) for s in range(NSLOT)]
    qt = [h.ap() for h in qth]
    bf16 = mybir.dt.bfloat16
    xT = nc.alloc_sbuf_tensor("xT", [BL, L], bf16).ap()
    ssp = nc.alloc_sbuf_tensor("ssp", [BL, NT], f32).ap()
    ssum = nc.alloc_sbuf_tensor("ssum", [BL, 1], f32).ap()
    rs = nc.alloc_sbuf_tensor("rs", [BL, 1], f32).ap()
    aT = nc.alloc_sbuf_tensor("aT", [BL, L], f32).ap()
    nmx = nc.alloc_sbuf_tensor("nmx_t", [BL, 1], f32).ap()

    # --- PSUM: two rotating energy banks
    ep = [nc.psum_tensor(f"ep{i}", [BL, TW], f32).__enter__().ap() for i in range(2)]

    # --- semaphores
    SW = nc.alloc_semaphore("SW")      # wst load
    SQQ = [[nc.alloc_semaphore(f"SQ{q}_{s}") for s in range(NSLOT)]
           for q in range(4)]
    SQE = [nc.alloc_semaphore(f"SQe{s}") for s in range(NSLOT)]
    SQF = nc.alloc_semaphore("SQf")  # last tile: strips 34..37
    SMM = nc.alloc_semaphore("SMM")    # PE tile done (1 per tile)
    SX = nc.alloc_semaphore("SX")      # ACT exp tile done
    SC = nc.alloc_semaphore("SC")      # nmx ready
    SRS = nc.alloc_semaphore("SRS")    # reciprocal ready
    SFIN = nc.alloc_semaphore("SFIN")   # DVE rescale half done
    SFIN2 = nc.alloc_semaphore("SFIN2")  # ACT rescale half done
    SNG = nc.alloc_semaphore("SNG")    # DVE same-engine ordering
    SOUT = nc.alloc_semaphore("SOUT")  # output DMA landed
    all_sems = [SW, *[s for qq in SQQ for s in qq], *SQE, SQF, SMM, SX, SC, SRS,
                SFIN, SFIN2, SNG, SOUT]
    sem_final = {s.name: 0 for s in all_sems}

    def inc(inst, sem, n=1):
        sem_final[sem.name] += n
        return inst.then_inc(sem, n)

    # DMA pattern for strips [s0, s1) of a tile: SBUF (partition q, strip s)
    # holds qvt row 128*s+q, cols l0..l0+w.
    def qv_tile_in(l0, w, s0, s1):
        return bass.AP(qvt_h, s0 * 128 * L + l0,
                       [[L, 128], [128 * L, s1 - s0], [1, w]])

    def emit_tile_dmas(sync, t, first=False):
        l0, w = TILES[t]
        s = t % NSLOT
        quarters = QUARTERS if t < NT - 1 else QUARTERS[:3] + [(31, 34)]
        for q, (s0, s1) in enumerate(quarters):
            inc(sync.dma_start(
                out=qt[s][:, s0 * w : s1 * w],
                in_=qv_tile_in(l0, w, s0, s1),
            ), SQQ[q][s], 16)
            if first and q == 0:
                # stationaries + shifts ride behind the first quarter so the
                # first tile's data leads the queue
                inc(sync.dma_start(out=wst, in_=wst_d), SW, 16)
                inc(sync.dma_start(out=nmx, in_=nmx_d), SC, 16)
        if t == NT - 1:
            inc(sync.dma_start(
                out=qt[s][:, 34 * w : 37 * w],
                in_=qv_tile_in(l0, w, 34, 37),
            ), SQF, 16)
        # 64-row final strip gets its own semaphore so the PE only blocks
        # on it at the very last matmul
        inc(sync.dma_start(
            out=bass.AP(qth[s], (NSTRIP - 1) * w, [[NSTRIP * w, LAST_ROWS], [1, w]]),
            in_=bass.AP(qvt_h, (NSTRIP - 1) * 128 * L + l0, [[L, LAST_ROWS], [1, w]]),
        ), SQE[s], 16)

    with nc.Block() as block:

        @block.sync
        def _(sync):
            # first tiles up front, then stream with slot reuse
            for t in range(NSLOT):
                emit_tile_dmas(sync, t, first=(t == 0))
            for t in range(NSLOT, NT):
                sync.wait_ge(SMM, t - NSLOT + 1)  # PE done with slot t%NSLOT
                emit_tile_dmas(sync, t)
            # stream the two rescaled output halves out as they finish
            sync.wait_ge(SFIN, 1)
            inc(sync.dma_start(out=out[:, :960], in_=aT[:, :960]), SOUT, 16)
            sync.wait_ge(SFIN2, 1)
            inc(sync.dma_start(out=out[:, 960:], in_=aT[:, 960:]), SOUT, 16)

        @block.tensor
        def _(pe):
            pe.wait_ge(SW, 16)
            for t in range(NT):
                l0, w = TILES[t]
                p = t // NSLOT
                if t >= 2:
                    pe.wait_ge(SX, t - 1)  # exp done with this ep bank
                for s in range(NSTRIP):
                    for q, (s0, s1) in enumerate(QUARTERS):
                        if s == s0:
                            pe.wait_ge(SQQ[q][t % NSLOT], 16 * p + 16)
                    if t == NT - 1 and s == 34:
                        pe.wait_ge(SQF, 16)
                    if s == NSTRIP - 1:
                        pe.wait_ge(SQE[t % NSLOT], 16 * p + 16)
                    rows = 128 if s < NSTRIP - 1 else LAST_ROWS
                    mm = pe.matmul(
                        ep[t % 2][:, :w],
                        wst[0:rows, s * BL : (s + 1) * BL],
                        bass.AP(qth[t % NSLOT], s * w, [[NSTRIP * w, rows], [1, w]]),
                        start=(s == 0),
                        stop=(s == NSTRIP - 1),
                    )
                inc(mm, SMM)

        @block.scalar
        def _(act):
            act.wait_ge(SC, 16)  # nmx loaded
            for t in range(NT):
                l0, w = TILES[t]
                act.wait_ge(SMM, t + 1)
                inc(act.activation(
                    xT[:, l0 : l0 + w], ep[t % 2][:, :w], Act.Exp,
                    bias=nmx, scale=1.0, accum_out=ssp[:, t : t + 1],
                ), SX)
            # tail: rescale second half once rs is ready
            act.wait_ge(SRS, 1)
            inc(act.mul(aT[:, 960:], xT[:, 960:], rs), SFIN2)

        @block.vector
        def _(dve):
            dve.wait_ge(SX, NT)  # all tiles exponentiated
            inc(dve.tensor_reduce(out=ssum, in_=ssp, axis=mybir.AxisListType.X,
                                  op=Alu.add), SNG)
            dve.wait_ge(SNG, 1)  # DVE deep pipeline: order ssum -> reciprocal
            inc(dve.reciprocal(rs, ssum), SRS)
            dve.wait_ge(SRS, 1)  # order rs -> rescale read
            inc(dve.tensor_scalar(out=aT[:, :960], in0=xT[:, :960],
                                  scalar1=rs, scalar2=None, op0=Alu.mult), SFIN)


        @block.gpsimd
        def _(gp):
            gp.wait_ge(SOUT, 32)

        nc.all_engine_barrier()
        for s in all_sems:
            if sem_final[s.name]:
                nc.gpsimd.sem_inc(s, -sem_final[s.name])

    return nc


def _get_nc():
    if "nc" not in _cache:
        _cache["nc"] = _build_nc()
    return _cache["nc"]


def make_in_maps(hidden, question_vector, W):
    hidden = np.asarray(hidden, dtype=np.float64)
    W = np.asarray(W, dtype=np.float64)
    qv = np.asarray(question_vector, dtype=np.float32)
    in_maps = []
    for i in range(NCORES):
        sl = slice(i * BL, (i + 1) * BL)
        wh = (W.T @ hidden[:, sl]).astype(np.float32)  # [H, BL]
        # block-sparse stationaries: wst[q, s*BL+b] = Wh[h, b] iff 128s+q = b*300+h
        wst = np.zeros((128, NSTRIP * BL), dtype=np.float16)
        r = np.arange(RROWS)
        bb, hh = r // H, r % H
        wst[r % 128, (r // 128) * BL + bb] = wh[hh, bb].astype(np.float16)
        sig = np.sqrt((wh.astype(np.float64) ** 2).sum(0))          # [BL]
        nmxv = -(MSCALE * sig + MOFF).astype(np.float32)[:, None]   # [BL, 1]
        # transposed qv: row (b,h), col l; padded to PROWS rows
        qs = qv[:, sl, :].astype(np.float16)           # [L, BL, H]
        qvt = qs.transpose(1, 2, 0).reshape(RROWS, L)
        in_maps.append({"qvt": np.ascontiguousarray(qvt), "wst": wst,
                        "nmx": np.ascontiguousarray(nmxv)})
    return in_maps


def kernel(hidden, question_vector, W, b=None, **kwargs):
    from concourse.bass_utils import run_bass_kernel_spmd

    nc = _get_nc()
    in_maps = make_in_maps(hidden, question_vector, W)
    res = run_bass_kernel_spmd(nc, in_maps, list(range(NCORES)))
    _cache["last_results"] = res
    outs = [np.asarray(res.results[i]["out"]) for i in range(NCORES)]
    attn = np.concatenate(outs, axis=0)[None]
    return np.ascontiguousarray(attn.astype(np.float32))
